# revision 1
# baseline (speedup 1.0000x reference)
import sys

sys.path.insert(0, "/opt/trn_rl_repo")

import numpy as np

N_GAUSS = 1024
IMG = 128
NB = 8          # gaussian blocks of 128
NP_ = 512       # pixels per matmul tile (one PSUM bank)
P_CORE = 2048   # pixels per core (16 rows x 128 cols)
N_CORES = 8

# packed-constants layout (free-dim offsets in the single (128, CONST_F) input)
O_BAS = N_GAUSS                 # basis rows 0..5
O_TRI = O_BAS + P_CORE          # strict-lower mask, 128 partitions
O_COL = O_TRI + 128             # colors lhsT, 128 x 24
O_ON1 = O_COL + 3 * NB          # ones row (partition 0, 128 wide)
O_ONC = O_ON1 + 128             # ones column (128 partitions, 1 wide)
CONST_F = O_ONC + 1

_prog_cache = {}


def _quat_to_rot(q):
    q = q / np.linalg.norm(q, axis=1, keepdims=True)
    w, x, y, z = q[:, 0], q[:, 1], q[:, 2], q[:, 3]
    R = np.stack([
        np.stack([1 - 2 * (y * y + z * z), 2 * (x * y - w * z), 2 * (x * z + w * y)], -1),
        np.stack([2 * (x * y + w * z), 1 - 2 * (x * x + z * z), 2 * (y * z - w * x)], -1),
        np.stack([2 * (x * z - w * y), 2 * (y * z + w * x), 1 - 2 * (x * x + y * y)], -1),
    ], -2)
    return R.astype(np.float32)


def _host_prep(camera_poses, positions, scales, rotations, opacity, features):
    pose = np.asarray(camera_poses, np.float32)[0]
    positions = np.asarray(positions, np.float32)
    scales = np.asarray(scales, np.float32)
    rotations = np.asarray(rotations, np.float32)
    opacity = np.asarray(opacity, np.float32)
    features = np.asarray(features, np.float32)
    N = positions.shape[0]

    hom = np.concatenate([positions, np.ones((N, 1), np.float32)], 1)      # (N,4)
    cam = hom @ pose.T                                                     # (N,4)
    depths = cam[:, 2]
    px = cam[:, 0] / depths
    py = cam[:, 1] / depths

    R = _quat_to_rot(rotations)                                            # (N,3,3)
    s2 = (scales * scales)[:, None, :]                                     # (N,1,3)
    cov3d = np.einsum('nij,nkj->nik', R * s2, R)                           # (N,3,3)

    x, y, z = cam[:, 0], cam[:, 1], depths
    zinv = 1.0 / z
    Jp = np.zeros((N, 2, 3), np.float32)
    Jp[:, 0, 0] = zinv
    Jp[:, 0, 2] = -x * zinv * zinv
    Jp[:, 1, 1] = zinv
    Jp[:, 1, 2] = -y * zinv * zinv
    Wc = pose[:3, :3]
    J = Jp @ Wc                                                            # (N,2,3)
    cov2d = np.einsum('nij,njk,nlk->nil', J, cov3d, J)                     # (N,2,2)

    a, b = cov2d[:, 0, 0], cov2d[:, 0, 1]
    c, d = cov2d[:, 1, 0], cov2d[:, 1, 1]
    det = a * d - b * c
    i00, i01, i10, i11 = d / det, -b / det, -c / det, a / det

    order = np.argsort(-depths, kind='stable')
    i00, i11 = i00[order], i11[order]
    s = (i01 + i10)[order]
    px, py = px[order], py[order]
    alp = np.maximum(opacity[order, 0], 1e-37)
    col = features[order]                                                  # (N,3)

    # logits = -0.5*m + ln(alpha) as quadratic in (gx, gy):
    #   A gx^2 + B gx gy + C gy^2 + D gx + E gy + F
    coeff = np.empty((6, N), np.float32)
    coeff[0] = -0.5 * i00
    coeff[1] = -0.5 * s
    coeff[2] = -0.5 * i11
    coeff[3] = i00 * px + 0.5 * s * py
    coeff[4] = 0.5 * s * px + i11 * py
    coeff[5] = -0.5 * (i00 * px * px + s * px * py + i11 * py * py) + np.log(alp)

    colT = np.zeros((128, 3 * NB), np.float32)
    for k in range(NB):
        colT[:, 3 * k:3 * k + 3] = col[k * 128:(k + 1) * 128]
    return coeff, colT


def _build_program():
    import concourse.bacc as bacc
    import concourse.mybir as mybir
    from concourse.tile import TileContext
    f32 = mybir.dt.float32
    EXP = mybir.ActivationFunctionType.Exp
    LN = mybir.ActivationFunctionType.Ln

    nc = bacc.Bacc("TRN2")
    consts_d = nc.dram_tensor("consts", (128, CONST_F), f32, kind="ExternalInput")
    out_d = nc.dram_tensor("out", (3, P_CORE), f32, kind="ExternalOutput")

    PT = P_CORE // NP_
    with TileContext(nc) as tc:
        with tc.tile_pool(name="const", bufs=1) as cpool, \
             tc.tile_pool(name="work", bufs=3) as wpool, \
             tc.tile_pool(name="carry", bufs=4) as crpool, \
             tc.tile_pool(name="outp", bufs=2) as opool, \
             tc.tile_pool(name="ps", bufs=2, space="PSUM") as pspool, \
             tc.tile_pool(name="psr", bufs=2, space="PSUM") as psr, \
             tc.tile_pool(name="psc", bufs=2, space="PSUM") as psc:
            consts = cpool.tile([128, CONST_F], f32)
            nc.sync.dma_start(out=consts[:, :], in_=consts_d[:, :])

            for pt in range(PT):
                carry = crpool.tile([1, NP_], f32, tag="carry")
                nc.vector.memset(carry[:, :], 0.0)
                rend = psr.tile([3, NP_], f32, tag="rend")
                for k in range(NB):
                    logits = pspool.tile([128, NP_], f32, tag="logits")
                    nc.tensor.matmul(out=logits[:, :],
                                     lhsT=consts[0:6, k * 128:(k + 1) * 128],
                                     rhs=consts[0:6, O_BAS + pt * NP_:O_BAS + (pt + 1) * NP_],
                                     start=True, stop=True)
                    am = wpool.tile([128, NP_], f32, tag="am")
                    nc.scalar.activation(out=am[:, :], in_=logits[:, :], func=EXP)
                    l1m = wpool.tile([128, NP_], f32, tag="l1m")
                    nc.scalar.activation(out=l1m[:, :], in_=am[:, :], func=LN,
                                         scale=-1.0, bias=1.0)
                    S = pspool.tile([128, NP_], f32, tag="S")
                    nc.tensor.matmul(out=S[:, :], lhsT=consts[0:1, O_ON1:O_ON1 + 128],
                                     rhs=carry[:, :], start=True, stop=False)
                    nc.tensor.matmul(out=S[:, :], lhsT=consts[0:128, O_TRI:O_TRI + 128],
                                     rhs=l1m[:, :], start=False, stop=True)
                    texcl = wpool.tile([128, NP_], f32, tag="texcl")
                    nc.scalar.activation(out=texcl[:, :], in_=S[:, :], func=EXP)
                    w = wpool.tile([128, NP_], f32, tag="w")
                    nc.vector.tensor_mul(out=w[:, :], in0=am[:, :], in1=texcl[:, :])
                    nc.tensor.matmul(out=rend[:, :],
                                     lhsT=consts[0:128, O_COL + 3 * k:O_COL + 3 * k + 3],
                                     rhs=w[:, :], start=(k == 0), stop=(k == NB - 1))
                    if k < NB - 1:
                        csum = psc.tile([1, NP_], f32, tag="csum")
                        nc.tensor.matmul(out=csum[:, :],
                                         lhsT=consts[0:128, O_ONC:O_ONC + 1],
                                         rhs=l1m[:, :], start=True, stop=True)
                        carry2 = crpool.tile([1, NP_], f32, tag="carry")
                        nc.vector.tensor_add(out=carry2[:, :], in0=carry[:, :],
                                             in1=csum[:, :])
                        carry = carry2
                ob = opool.tile([3, NP_], f32, tag="ob")
                nc.vector.tensor_copy(out=ob[:, :], in_=rend[:, :])
                nc.sync.dma_start(out=out_d[:, pt * NP_:(pt + 1) * NP_], in_=ob[:, :])
    nc.finalize()
    return nc


def _get_prog():
    if "nc" not in _prog_cache:
        _prog_cache["nc"] = _build_program()
    return _prog_cache["nc"]


def _make_in_maps(coeff, colT):
    ys = np.linspace(-1.0, 1.0, IMG, dtype=np.float32)
    xs = np.linspace(-1.0, 1.0, IMG, dtype=np.float32)
    rows_per_core = IMG // N_CORES
    in_maps = []
    for c in range(N_CORES):
        gy = np.repeat(ys[c * rows_per_core:(c + 1) * rows_per_core], IMG)
        gx = np.tile(xs, rows_per_core)
        basis = np.stack([gx * gx, gx * gy, gy * gy, gx, gy,
                          np.ones_like(gx)]).astype(np.float32)
        consts = np.zeros((128, CONST_F), np.float32)
        consts[0:6, 0:N_GAUSS] = coeff
        consts[0:6, O_BAS:O_BAS + P_CORE] = basis
        consts[:, O_TRI:O_TRI + 128] = np.triu(np.ones((128, 128), np.float32), 1)
        consts[:, O_COL:O_COL + 3 * NB] = colT
        consts[0, O_ON1:O_ON1 + 128] = 1.0
        consts[:, O_ONC] = 1.0
        in_maps.append({"consts": consts})
    return in_maps


def _run(inputs, trace=False):
    from concourse.bass_utils import run_bass_kernel_spmd
    coeff, colT = _host_prep(inputs["camera_poses"], inputs["positions"],
                             inputs["scales"], inputs["rotations"],
                             inputs["opacity"], inputs["features"])
    nc = _get_prog()
    in_maps = _make_in_maps(coeff, colT)
    res = run_bass_kernel_spmd(nc, in_maps, core_ids=list(range(N_CORES)),
                               trace=trace)
    rows_per_core = IMG // N_CORES
    out = np.zeros((1, 3, IMG, IMG), np.float32)
    for c in range(N_CORES):
        out[0, :, c * rows_per_core:(c + 1) * rows_per_core, :] = \
            res.results[c]["out"].reshape(3, rows_per_core, IMG)
    return out, res


def kernel(camera_poses, positions, scales, rotations, opacity, features, H, W):
    assert int(H) == IMG and int(W) == IMG
    out, _ = _run({"camera_poses": camera_poses, "positions": positions,
                   "scales": scales, "rotations": rotations, "opacity": opacity,
                   "features": features})
    return out



# revision 4
# speedup vs baseline: 6.5702x; 6.5702x over previous
import sys

sys.path.insert(0, "/opt/trn_rl_repo")

import numpy as np

N_GAUSS = 1024
IMG = 128
NB = 8          # gaussian blocks of 128
NP_ = 512       # pixels per matmul tile (one PSUM bank)
P_CORE = 2048   # pixels per core (16 rows x 128 cols)
N_CORES = 8
PT = P_CORE // NP_

_cache = {}


def _quat_to_rot(q):
    q = q / np.linalg.norm(q, axis=1, keepdims=True)
    w, x, y, z = q[:, 0], q[:, 1], q[:, 2], q[:, 3]
    R = np.stack([
        np.stack([1 - 2 * (y * y + z * z), 2 * (x * y - w * z), 2 * (x * z + w * y)], -1),
        np.stack([2 * (x * y + w * z), 1 - 2 * (x * x + z * z), 2 * (y * z - w * x)], -1),
        np.stack([2 * (x * z - w * y), 2 * (y * z + w * x), 1 - 2 * (x * x + y * y)], -1),
    ], -2)
    return R.astype(np.float32)


def _host_prep(camera_poses, positions, scales, rotations, opacity, features):
    pose = np.asarray(camera_poses, np.float32)[0]
    positions = np.asarray(positions, np.float32)
    scales = np.asarray(scales, np.float32)
    rotations = np.asarray(rotations, np.float32)
    opacity = np.asarray(opacity, np.float32)
    features = np.asarray(features, np.float32)
    N = positions.shape[0]

    hom = np.concatenate([positions, np.ones((N, 1), np.float32)], 1)      # (N,4)
    cam = hom @ pose.T                                                     # (N,4)
    depths = cam[:, 2]
    px = cam[:, 0] / depths
    py = cam[:, 1] / depths

    R = _quat_to_rot(rotations)                                            # (N,3,3)
    s2 = (scales * scales)[:, None, :]                                     # (N,1,3)
    cov3d = np.einsum('nij,nkj->nik', R * s2, R)                           # (N,3,3)

    x, y, z = cam[:, 0], cam[:, 1], depths
    zinv = 1.0 / z
    Jp = np.zeros((N, 2, 3), np.float32)
    Jp[:, 0, 0] = zinv
    Jp[:, 0, 2] = -x * zinv * zinv
    Jp[:, 1, 1] = zinv
    Jp[:, 1, 2] = -y * zinv * zinv
    Wc = pose[:3, :3]
    J = Jp @ Wc                                                            # (N,2,3)
    cov2d = np.einsum('nij,njk,nlk->nil', J, cov3d, J)                     # (N,2,2)

    a, b = cov2d[:, 0, 0], cov2d[:, 0, 1]
    c, d = cov2d[:, 1, 0], cov2d[:, 1, 1]
    det = a * d - b * c
    i00, i01, i10, i11 = d / det, -b / det, -c / det, a / det

    order = np.argsort(-depths, kind='stable')
    i00, i11 = i00[order], i11[order]
    s = (i01 + i10)[order]
    px, py = px[order], py[order]
    alp = np.maximum(opacity[order, 0], 1e-37)
    col = features[order]                                                  # (N,3)

    # logits = -0.5*m + ln(alpha) as quadratic in (gx, gy_local):
    #   A gx^2 + B gx t + C t^2 + D gx + E t + F   with gy = u_core + t.
    # Each core renders 16 image rows; fold its y-offset u into the
    # gaussian center so the on-device pixel basis is core-invariant.
    ys = np.linspace(-1.0, 1.0, IMG, dtype=np.float32)
    u = ys[::IMG // N_CORES][:, None]                                      # (8,1)
    pyc = py[None, :] - u                                                  # (8,N)
    lna = np.log(alp)
    coeff8 = np.empty((N_CORES, 6, N), np.float32)
    coeff8[:, 0] = -0.5 * i00
    coeff8[:, 1] = -0.5 * s
    coeff8[:, 2] = -0.5 * i11
    coeff8[:, 3] = i00 * px + 0.5 * s * pyc
    coeff8[:, 4] = 0.5 * s * px + i11 * pyc
    coeff8[:, 5] = -0.5 * (i00 * px * px + s * px * pyc + i11 * pyc * pyc) + lna

    colT = np.zeros((128, 3 * NB), np.float32)
    for k in range(NB):
        colT[:, 3 * k:3 * k + 3] = col[k * 128:(k + 1) * 128]
    return coeff8, colT


def _build_program():
    import concourse.bacc as bacc
    import concourse.mybir as mybir
    from concourse.tile import TileContext
    f32 = mybir.dt.float32
    EXP = mybir.ActivationFunctionType.Exp
    LN = mybir.ActivationFunctionType.Ln

    nc = bacc.Bacc("TRN2")
    coeff_d = nc.dram_tensor("coeff", (6, N_GAUSS), f32, kind="ExternalInput")
    colt_d = nc.dram_tensor("colt", (128, 3 * NB), f32, kind="ExternalInput")
    out_d = nc.dram_tensor("out", (3, P_CORE), f32, kind="ExternalOutput")

    # Call-invariant data rides in the NEFF (loaded to HBM once at model
    # load) instead of being shipped per call.
    xs = np.linspace(-1.0, 1.0, IMG).astype(np.float32)
    rows = IMG // N_CORES
    gx = np.tile(xs, rows)
    gy = np.repeat((np.arange(rows) * (2.0 / (IMG - 1))).astype(np.float32), IMG)
    basis = np.stack([gx * gx, gx * gy, gy * gy, gx, gy,
                      np.ones_like(gx)]).astype(np.float32)                # (6,2048)
    basis_d = nc.inline_tensor(np.ascontiguousarray(basis), "basis")
    tri_d = nc.inline_tensor(np.triu(np.ones((128, 128), np.float32), 1), "tri")
    onesrow_d = nc.inline_tensor(np.ones((1, 128), np.float32), "onesrow")
    onescol_d = nc.inline_tensor(np.ones((128, 1), np.float32), "onescol")

    with TileContext(nc) as tc:
        with tc.tile_pool(name="const", bufs=1) as cpool, \
             tc.tile_pool(name="work", bufs=3) as wpool, \
             tc.tile_pool(name="carry", bufs=4) as crpool, \
             tc.tile_pool(name="outp", bufs=2) as opool, \
             tc.tile_pool(name="ps", bufs=2, space="PSUM") as pspool, \
             tc.tile_pool(name="psr", bufs=2, space="PSUM") as psr, \
             tc.tile_pool(name="psc", bufs=2, space="PSUM") as psc:
            coeff = cpool.tile([6, N_GAUSS], f32)
            nc.sync.dma_start(out=coeff[:, :], in_=coeff_d[:, :])
            colt = cpool.tile([128, 3 * NB], f32)
            nc.sync.dma_start(out=colt[:, :], in_=colt_d[:, :])
            bas = cpool.tile([6, P_CORE], f32)
            nc.sync.dma_start(out=bas[:, :], in_=basis_d[:, :])
            tri = cpool.tile([128, 128], f32)
            nc.sync.dma_start(out=tri[:, :], in_=tri_d[:, :])
            onr = cpool.tile([1, 128], f32)
            nc.sync.dma_start(out=onr[:, :], in_=onesrow_d[:, :])
            onc = cpool.tile([128, 1], f32)
            nc.sync.dma_start(out=onc[:, :], in_=onescol_d[:, :])

            for pt in range(PT):
                carry = crpool.tile([1, NP_], f32, tag="carry")
                nc.vector.memset(carry[:, :], 0.0)
                rend = psr.tile([3, NP_], f32, tag="rend")
                for k in range(NB):
                    logits = pspool.tile([128, NP_], f32, tag="logits")
                    nc.tensor.matmul(out=logits[:, :],
                                     lhsT=coeff[0:6, k * 128:(k + 1) * 128],
                                     rhs=bas[0:6, pt * NP_:(pt + 1) * NP_],
                                     start=True, stop=True)
                    am = wpool.tile([128, NP_], f32, tag="am")
                    nc.scalar.activation(out=am[:, :], in_=logits[:, :], func=EXP)
                    l1m = wpool.tile([128, NP_], f32, tag="l1m")
                    nc.scalar.activation(out=l1m[:, :], in_=am[:, :], func=LN,
                                         scale=-1.0, bias=1.0)
                    S = pspool.tile([128, NP_], f32, tag="S")
                    nc.tensor.matmul(out=S[:, :], lhsT=onr[0:1, 0:128],
                                     rhs=carry[:, :], start=True, stop=False)
                    nc.tensor.matmul(out=S[:, :], lhsT=tri[0:128, 0:128],
                                     rhs=l1m[:, :], start=False, stop=True)
                    texcl = wpool.tile([128, NP_], f32, tag="texcl")
                    nc.scalar.activation(out=texcl[:, :], in_=S[:, :], func=EXP)
                    w = wpool.tile([128, NP_], f32, tag="w")
                    nc.vector.tensor_mul(out=w[:, :], in0=am[:, :], in1=texcl[:, :])
                    nc.tensor.matmul(out=rend[:, :],
                                     lhsT=colt[0:128, 3 * k:3 * k + 3],
                                     rhs=w[:, :], start=(k == 0), stop=(k == NB - 1))
                    if k < NB - 1:
                        csum = psc.tile([1, NP_], f32, tag="csum")
                        nc.tensor.matmul(out=csum[:, :],
                                         lhsT=onc[0:128, 0:1],
                                         rhs=l1m[:, :], start=True, stop=True)
                        carry2 = crpool.tile([1, NP_], f32, tag="carry")
                        nc.vector.tensor_add(out=carry2[:, :], in0=carry[:, :],
                                             in1=csum[:, :])
                        carry = carry2
                ob = opool.tile([3, NP_], f32, tag="ob")
                nc.vector.tensor_copy(out=ob[:, :], in_=rend[:, :])
                nc.sync.dma_start(out=out_d[:, pt * NP_:(pt + 1) * NP_], in_=ob[:, :])
    nc.finalize()
    return nc


def _get_runner():
    """Build the Bass program and a persistently cached jitted executor.

    Mirrors concourse.bass2jax.run_bass_via_pjrt's multi-core path, but the
    jit-wrapped shard_map closure is created ONCE and reused — the library
    rebuilds it per call, which re-traces and re-dispatches the executable
    on every invocation.
    """
    if "runner" in _cache:
        return _cache["runner"]
    import jax
    from jax.experimental.shard_map import shard_map
    from jax.sharding import Mesh, PartitionSpec
    import concourse.mybir as mybir
    from concourse import bass2jax

    bass2jax.install_neuronx_cc_hook()
    nc = _build_program()
    assert nc.dbg_addr is None and not nc.dbg_callbacks
    partition_name = nc.partition_id_tensor.name if nc.partition_id_tensor else None

    in_names, out_names, out_avals = [], [], []
    for alloc in nc.m.functions[0].allocations:
        if not isinstance(alloc, mybir.MemoryLocationSet):
            continue
        name = alloc.memorylocations[0].name
        if alloc.kind == "ExternalInput":
            if name != partition_name:
                in_names.append(name)
        elif alloc.kind == "ExternalOutput":
            shape = tuple(alloc.tensor_shape)
            dtype = mybir.dt.np(alloc.dtype)
            out_names.append(name)
            out_avals.append(jax.core.ShapedArray(shape, dtype))
    n_params = len(in_names)
    n_outs = len(out_avals)
    all_in_names = tuple(in_names + out_names
                         + ([partition_name] if partition_name else []))
    donate = tuple(range(n_params, n_params + n_outs))

    def _body(*args):
        operands = list(args)
        if partition_name is not None:
            operands.append(bass2jax.partition_id_tensor())
        outs = bass2jax._bass_exec_p.bind(
            *operands,
            out_avals=tuple(out_avals),
            in_names=all_in_names,
            out_names=tuple(out_names),
            lowering_input_output_aliases=(),
            sim_require_finite=True,
            sim_require_nnan=True,
            nc=nc,
        )
        return tuple(outs)

    devices = jax.devices()[:N_CORES]
    assert len(devices) == N_CORES
    mesh = Mesh(np.asarray(devices), ("core",))
    in_specs = (PartitionSpec("core"),) * (n_params + n_outs)
    out_specs = (PartitionSpec("core"),) * n_outs
    sharded = jax.jit(
        shard_map(_body, mesh=mesh, in_specs=in_specs, out_specs=out_specs,
                  check_rep=False),
        donate_argnums=donate, keep_unused=True,
    )
    _cache["runner"] = (sharded, in_names, out_names, out_avals)
    return _cache["runner"]


def _run(inputs):
    coeff8, colT = _host_prep(inputs["camera_poses"], inputs["positions"],
                              inputs["scales"], inputs["rotations"],
                              inputs["opacity"], inputs["features"])
    sharded, in_names, out_names, out_avals = _get_runner()
    per_input = {
        "coeff": np.ascontiguousarray(coeff8.reshape(N_CORES * 6, N_GAUSS)),
        "colt": np.ascontiguousarray(np.tile(colT, (N_CORES, 1))),
    }
    concat_in = [per_input[name] for name in in_names]
    concat_zeros = [np.zeros((N_CORES * a.shape[0], *a.shape[1:]), a.dtype)
                    for a in out_avals]
    out_arrs = sharded(*concat_in, *concat_zeros)
    res = np.asarray(out_arrs[out_names.index("out")]).reshape(
        N_CORES, 3, P_CORE)
    rows = IMG // N_CORES
    out = np.zeros((1, 3, IMG, IMG), np.float32)
    for c in range(N_CORES):
        out[0, :, c * rows:(c + 1) * rows, :] = res[c].reshape(3, rows, IMG)
    return out


def kernel(camera_poses, positions, scales, rotations, opacity, features, H, W):
    assert int(H) == IMG and int(W) == IMG
    return _run({"camera_poses": camera_poses, "positions": positions,
                 "scales": scales, "rotations": rotations, "opacity": opacity,
                 "features": features})
